# revision 18
# baseline (speedup 1.0000x reference)
"""Trainium2 Bass kernel for nn_BERT_89017492177566 (sparse sliding-window BERT).

Data-parallel over batch: 8 sequences -> 8 NeuronCores, one sequence each.
Activations kept feature-major (FM: features on partitions, tokens on free axis)
in SBUF; all big matmuls bf16 with f32 PSUM accumulation. LayerNorm scale/bias
and the 1/sqrt(hd) query scale are folded into the weights on the host.
Banded attention (|q-k| <= 128) computed per 128-query block against a 384-wide
key window; softmax weights are produced in both orientations (token-major for
row sums + the all_w output, transposed for the AV matmul).
"""

import os
import numpy as np
import ml_dtypes

import concourse.bass as bass
import concourse.bacc as bacc
import concourse.mybir as mybir
import concourse.tile as tile
from concourse.bass_utils import run_bass_kernel_spmd

BF16 = mybir.dt.bfloat16
F32 = mybir.dt.float32
F32R = mybir.dt.float32r
I32 = mybir.dt.int32

H = 12
EPS = 1e-5
S = 1024
D = 768
F = 3072
L = 4
B = 8
NB = S // 128           # 8 token blocks
DC = D // 128           # 6 feature chunks
FC = F // 128           # 24 ff chunks
W = 3 * 128             # key window width

nbf = ml_dtypes.bfloat16

_CACHE = {}
LAST_RESULTS = None


def _build_nc(apply_biases):
    """Emit the full Bass/Tile program. apply_biases: dict of bools per bias group."""
    nc = bacc.Bacc("TRN2", target_bir_lowering=False, debug=False, num_devices=8)

    # ---------------- DRAM tensors ----------------
    emb_d = nc.dram_tensor("emb", [32000, D], F32, kind="ExternalInput")
    tok_d = nc.dram_tensor("tokens", [S], I32, kind="ExternalInput")
    pen_d = nc.dram_tensor("penalty", [1, S], F32, kind="ExternalInput")
    pet_d = nc.dram_tensor("pe_t", [DC, 128, S], F32, kind="ExternalInput")
    mtm_d = nc.dram_tensor("mask_tm", [NB, 128, W], BF16, kind="ExternalInput")
    mtt_d = nc.dram_tensor("mask_tt", [NB, 3, 128, 128], BF16, kind="ExternalInput")
    idf_d = nc.dram_tensor("ident_f32", [128, 128], F32, kind="ExternalInput")
    idb_d = nc.dram_tensor("ident_bf16", [128, 128], BF16, kind="ExternalInput")

    wqk_d = nc.dram_tensor("wqk_t", [L, 12, 128, DC, 128], BF16, kind="ExternalInput")
    wv_d = nc.dram_tensor("wv_t", [L, 2, 128, DC, 384], BF16, kind="ExternalInput")
    wo_d = nc.dram_tensor("wo_t", [L, DC, 128, DC, 128], BF16, kind="ExternalInput")
    w1_d = nc.dram_tensor("w1_t", [L, FC, 128, DC, 128], BF16, kind="ExternalInput")
    w2_d = nc.dram_tensor("w2_t", [L, DC, 128, FC, 128], BF16, kind="ExternalInput")
    wc1_d = nc.dram_tensor("wc1_t", [DC, 128, DC, 128], BF16, kind="ExternalInput")
    wc2_d = nc.dram_tensor("wc2_t", [DC, 128, 2], BF16, kind="ExternalInput")
    wp_d = nc.dram_tensor("wp_t", [128, DC], BF16, kind="ExternalInput")

    bias_d = {}
    for key, shp, dt_ in (
        ("qk", [L, 1, 1536], BF16), ("v", [L, 1, D], BF16), ("o", [L, 1, D], BF16),
        ("f1", [L, 1, F], BF16), ("f2", [L, 1, D], BF16),
        ("c1", [1, D], BF16), ("c2", [2, 1], F32),
    ):
        if apply_biases[key]:
            bias_d[key] = nc.dram_tensor("b_" + key, shp, dt_, kind="ExternalInput")

    out_d = nc.dram_tensor("cls_out", [2], F32, kind="ExternalOutput")
    aw_d = nc.dram_tensor("all_w", [L, S, S], F32, kind="ExternalOutput")

    AX = mybir.AxisListType.X
    OP = mybir.AluOpType
    AF = mybir.ActivationFunctionType

    def mm_seq(ps, pairs):
        n = len(pairs)
        for i, (lt, rh) in enumerate(pairs):
            nc.tensor.matmul(ps, lt, rh, start=(i == 0), stop=(i == n - 1))

    with tile.TileContext(nc) as tc:
        with (
            tc.tile_pool(name="const", bufs=1) as cpool,
            tc.tile_pool(name="state", bufs=1) as spool,
        ):
            # ---------------- constants ----------------
            idf = cpool.tile([128, 128], F32)
            nc.sync.dma_start(idf[:], idf_d.ap())
            idb = cpool.tile([128, 128], BF16)
            nc.sync.dma_start(idb[:], idb_d.ap())
            mtm = cpool.tile([128, NB, W], BF16)
            nc.sync.dma_start(mtm[:], mtm_d.ap().rearrange("b p w -> p b w"))
            mtt = cpool.tile([128, NB, 3, 128], BF16)
            nc.sync.dma_start(mtt[:], mtt_d.ap().rearrange("b j p q -> p b j q"))
            idx = cpool.tile([128, NB], I32)
            nc.sync.dma_start(idx[:], tok_d.ap().rearrange("(b p) -> p b", p=128))
            pen = cpool.tile([1, S], F32)
            nc.sync.dma_start(pen[:], pen_d.ap())
            wp = cpool.tile([128, DC], BF16)
            nc.sync.dma_start(wp[:], wp_d.ap())
            wc2 = cpool.tile([128, DC, 2], BF16)
            nc.sync.dma_start(wc2[:], wc2_d.ap().rearrange("c p t -> p c t"))
            ones_col = cpool.tile([128, 1], BF16)
            nc.gpsimd.memset(ones_col[:], 1.0)
            ones_row = cpool.tile([1, 512], BF16)
            nc.gpsimd.memset(ones_row[:], 1.0)
            epsc = cpool.tile([128, 1], F32)
            nc.gpsimd.memset(epsc[:], EPS)
            btiles = {}
            for key in ("qk", "v", "o", "f1", "f2"):
                if key in bias_d:
                    n_ = bias_d[key].shape[2]
                    t = cpool.tile([1, L * n_], BF16, name=f"bt_{key}")
                    nc.sync.dma_start(
                        t[:], bias_d[key].ap().rearrange("l o n -> o (l n)")
                    )
                    btiles[key] = (t, n_)
            if "c1" in bias_d:
                bc1 = cpool.tile([1, D], BF16)
                nc.sync.dma_start(bc1[:], bias_d["c1"].ap())
            if "c2" in bias_d:
                bc2 = cpool.tile([2, 1], F32)
                nc.sync.dma_start(bc2[:], bias_d["c2"].ap())

            def bias_pair(key, l, off, width, nrep, fm=True):
                """Rank-1 bias matmul operands, or None. fm: bias on partitions."""
                if key not in btiles:
                    return None
                row, n_ = btiles[key]
                base = l * n_
                sl = row[:, base + off : base + off + width]
                if fm:
                    return (sl, ones_row[:, :nrep])
                return (ones_row[:, :nrep], sl)

            # ---------------- persistent state ----------------
            x = spool.tile([128, DC, S], F32)           # residual, feature-major
            h = spool.tile([128, DC, S], BF16)          # LN1/LN2 output (reused)
            h2 = h
            qT = spool.tile([128, DC, S], BF16)
            kTp = spool.tile([128, DC, S + 256], BF16)  # padded keys
            v_tm = spool.tile([128, NB, D], BF16)       # token-major values
            attnF = spool.tile([128, DC, S], BF16)      # attention out, feature-major
            ff1 = spool.tile([128, FC, 512], BF16)      # one token-half at a time
            s1row = spool.tile([1, S], F32)   # stat sums (partition 0)
            s2row = spool.tile([1, S], F32)
            bcA = spool.tile([128, S], F32)   # sum -> mean -> mean*rstd
            bcB = spool.tile([128, S], F32)   # sumsq -> var -> sd
            r_bc = spool.tile([128, S], F32)  # scratch -> rstd (broadcast)
            pooled = spool.tile([128, DC], F32)
            hc = spool.tile([128, DC], BF16)
            c1 = spool.tile([128, DC], BF16)

            # zero the key pads once; never written again
            nc.gpsimd.memset(kTp[:, :, 0:128], 0.0)
            nc.gpsimd.memset(kTp[:, :, S + 128 : S + 256], 0.0)

            # ---------------- embedding + positional ----------------
            with (
                tc.tile_pool(name="embed", bufs=2) as epool,
                tc.tile_pool(name="embps", bufs=2, space="PSUM") as eps,
            ):
                # absorb identity-DMA waits into PE's clock (transpose-mode
                # LDWEIGHTS supports only one sync wait slot)
                ps_ab = eps.tile([128, 128], F32, tag="tr", name="ps_ab")
                nc.tensor.transpose(ps_ab[:], idf[:], idf[:])
                ps_abb = eps.tile([128, 128], BF16, tag="trb", name="ps_abb")
                nc.tensor.transpose(ps_abb[:], idb[:], idb[:])
                pe_c = []
                for c in range(DC):
                    t = epool.tile([128, S], F32, tag=f"pe{c}", bufs=1, name=f"pe{c}")
                    nc.sync.dma_start(t[:], pet_d.ap()[c])
                    pe_c.append(t)
                for tb in range(NB):
                    x0 = epool.tile([128, D], F32, tag="x0", name="x0")
                    nc.gpsimd.indirect_dma_start(
                        out=x0[:],
                        out_offset=None,
                        in_=emb_d.ap(),
                        in_offset=bass.IndirectOffsetOnAxis(
                            ap=idx[:, tb : tb + 1], axis=0
                        ),
                    )
                    # bounce through DVE so every transpose dep is in the DVE
                    # sem domain (single wait on the PE side)
                    x0b = epool.tile([128, D], F32, tag="x0b", name="x0b")
                    nc.vector.tensor_copy(x0b[:], x0[:])
                    for c in range(DC):
                        ps = eps.tile([128, 128], F32, tag="tr", name="ps_tr")
                        nc.tensor.transpose(ps[:], x0b[:, 128 * c : 128 * (c + 1)], idf[:])
                        nc.vector.tensor_tensor(
                            out=x[:, c, 128 * tb : 128 * (tb + 1)],
                            in0=ps[:],
                            in1=pe_c[c][:, 128 * tb : 128 * (tb + 1)],
                            op=OP.add,
                        )

            # ---------------- main work pools ----------------
            with (
                tc.tile_pool(name="wts", bufs=1) as wpool,
                tc.tile_pool(name="work", bufs=1) as wk,
                tc.tile_pool(name="atw", bufs=3) as aw,
                tc.tile_pool(name="ps", bufs=1, space="PSUM") as pp,
            ):

                def layer_norm(src, dst):
                    """dst(bf16) = (src - mean) * rstd, feature-major."""
                    for half in range(2):
                        sl = slice(512 * half, 512 * (half + 1))
                        # sums of x (bf16 casts just-in-time)
                        ps = pp.tile([1, 512], F32, tag="mm512", bufs=2, name="ps_st")
                        xbs = []
                        for c in range(DC):
                            xb = wk.tile([128, 512], BF16, tag=f"xb{c % 3}",
                                         bufs=2, name="xb")
                            nc.vector.tensor_copy(xb[:], src[:, c, sl])
                            xbs.append(xb)
                            nc.tensor.matmul(
                                ps[:], ones_col[:], xb[:],
                                start=(c == 0), stop=(c == DC - 1),
                            )
                        nc.scalar.activation(s1row[:, sl], ps[:], AF.Copy)
                        # sums of x^2
                        ps2 = pp.tile([1, 512], F32, tag="mm512", bufs=2, name="ps_st2")
                        for c in range(DC):
                            xsq = wk.tile([128, 512], BF16, tag="xsq", bufs=2, name="xsq")
                            nc.vector.tensor_tensor(
                                out=xsq[:], in0=src[:, c, sl], in1=src[:, c, sl],
                                op=OP.mult,
                            )
                            nc.tensor.matmul(
                                ps2[:], ones_col[:], xsq[:],
                                start=(c == 0), stop=(c == DC - 1),
                            )
                        nc.scalar.activation(s2row[:, sl], ps2[:], AF.Copy)
                    # broadcast sums to all partitions, then full-width math:
                    # bcA: mean -> mean*rstd ; bcB: var -> sd ; r_bc: rstd
                    nc.gpsimd.partition_broadcast(bcA[:], s1row[:])
                    nc.gpsimd.partition_broadcast(bcB[:], s2row[:])
                    nc.vector.tensor_scalar(
                        out=bcA[:], in0=bcA[:], scalar1=1.0 / D, scalar2=None,
                        op0=OP.mult,
                    )
                    nc.vector.tensor_tensor(
                        out=r_bc[:], in0=bcA[:], in1=bcA[:], op=OP.mult
                    )
                    nc.vector.tensor_scalar(
                        out=bcB[:], in0=bcB[:], scalar1=1.0 / D, scalar2=None,
                        op0=OP.mult,
                    )
                    nc.vector.tensor_tensor(
                        out=bcB[:], in0=bcB[:], in1=r_bc[:], op=OP.subtract
                    )
                    nc.scalar.activation(bcB[:], bcB[:], AF.Sqrt, bias=epsc[:])
                    nc.vector.reciprocal(r_bc[:], bcB[:])
                    nc.vector.tensor_tensor(
                        out=bcA[:], in0=bcA[:], in1=r_bc[:], op=OP.mult
                    )
                    for c in range(DC):
                        tmp = wk.tile([128, S], F32, tag="lntmp", bufs=2, name="lntmp")
                        nc.vector.tensor_tensor(
                            out=tmp[:], in0=src[:, c, :], in1=r_bc[:], op=OP.mult
                        )
                        nc.vector.tensor_tensor(
                            out=dst[:, c, :], in0=tmp[:], in1=bcA[:], op=OP.subtract
                        )

                for l in range(L):
                    # ======== LN1 ========
                    layer_norm(x, h)

                    # ======== QKV: q,k feature-major ========
                    for mo in range(12):
                        wt = wpool.tile([128, DC, 128], BF16, tag="wqk",
                                        bufs=3, name="wqk")
                        nc.sync.dma_start(wt[:], wqk_d.ap()[l, mo])
                        for no in range(2):
                            sl = slice(512 * no, 512 * (no + 1))
                            ps = pp.tile([128, 512], F32, tag="mm512", bufs=2, name="ps_qk")
                            pairs = [(wt[:, kc, :], h[:, kc, sl]) for kc in range(DC)]
                            bp = bias_pair("qk", l, 128 * mo, 128, 512)
                            if bp:
                                pairs.append(bp)
                            mm_seq(ps[:], pairs)
                            if mo < 6:
                                nc.scalar.activation(qT[:, mo, sl], ps[:], AF.Copy)
                            else:
                                nc.scalar.activation(
                                    kTp[:, mo - 6,
                                        slice(128 + 512 * no, 128 + 512 * (no + 1))],
                                    ps[:], AF.Copy,
                                )

                    # ======== V token-major ========
                    for no in range(2):
                        wv = wpool.tile([128, DC, 384], BF16, tag="wv",
                                        bufs=1, name="wv")
                        nc.sync.dma_start(wv[:], wv_d.ap()[l, no])
                        for tb in range(NB):
                            ps = pp.tile([128, 384], F32, tag="s_tm", bufs=2, name="ps_v")
                            pairs = [
                                (h[:, kc, 128 * tb : 128 * (tb + 1)], wv[:, kc, :])
                                for kc in range(DC)
                            ]
                            bp = bias_pair("v", l, 384 * no, 384, 128, fm=False)
                            if bp:
                                pairs.append(bp)
                            mm_seq(ps[:], pairs)
                            nc.scalar.activation(
                                v_tm[:, tb, 384 * no : 384 * (no + 1)], ps[:], AF.Copy
                            )

                    # ======== banded attention ========
                    for b in range(NB):
                        acc = wk.tile([128, W], F32, tag=f"acc{b & 1}", name="acc")
                        at_tm = wk.tile([128, D], BF16, tag=f"attm{b & 1}", name="at_tm")
                        vjs = [j for j in range(3) if 0 <= b - 1 + j <= 7]
                        for hh in range(H):
                            c_h, o_h = (64 * hh) // 128, (64 * hh) % 128
                            q_h = qT[o_h : o_h + 64, c_h, 128 * b : 128 * (b + 1)]
                            ps_s = pp.tile([128, W], F32, tag="s_tm", bufs=2, name="ps_s")
                            nc.tensor.matmul(
                                ps_s[:], q_h,
                                kTp[o_h : o_h + 64, c_h, 128 * b : 128 * b + W],
                                start=True, stop=False,
                            )
                            nc.tensor.matmul(
                                ps_s[:], idb[:], mtm[:, b, :],
                                start=False, stop=True,
                            )
                            ps_t = pp.tile([128, W], F32, tag="sT", bufs=2, name="ps_t")
                            for j in range(3):
                                nc.tensor.matmul(
                                    ps_t[:, 128 * j : 128 * (j + 1)],
                                    kTp[o_h : o_h + 64, c_h,
                                        128 * (b + j) : 128 * (b + j + 1)],
                                    q_h,
                                    start=True, stop=False,
                                )
                                nc.tensor.matmul(
                                    ps_t[:, 128 * j : 128 * (j + 1)],
                                    idb[:], mtt[:, b, j, :],
                                    start=False, stop=True,
                                )
                            ssum = aw.tile([128, 1], F32, tag="ssum", name="ssum")
                            wtm = aw.tile([128, W], BF16, tag="wtm", name="wtm")
                            nc.scalar.activation(
                                wtm[:], ps_s[:], AF.Exp, accum_out=ssum[:]
                            )
                            wtt = aw.tile([128, W], BF16, tag="wtt", name="wtt")
                            nc.scalar.activation(wtt[:], ps_t[:], AF.Exp)
                            rr = aw.tile([128, 1], F32, tag="rr", name="rr")
                            nc.vector.reciprocal(rr[:], ssum[:])
                            # all_w accumulation: acc (+)= wtm * rr / H
                            if hh == 0:
                                nc.vector.tensor_scalar(
                                    out=acc[:], in0=wtm[:], scalar1=rr[:],
                                    scalar2=1.0 / H, op0=OP.mult, op1=OP.mult,
                                )
                            else:
                                tmp = aw.tile([128, W], F32, tag="awtmp", name="awtmp")
                                nc.vector.tensor_scalar(
                                    out=tmp[:], in0=wtm[:], scalar1=rr[:],
                                    scalar2=1.0 / H, op0=OP.mult, op1=OP.mult,
                                )
                                nc.gpsimd.tensor_tensor(
                                    out=acc[:], in0=acc[:], in1=tmp[:], op=OP.add
                                )
                            # AV -> token-major attn head, normalized at evac
                            ps_av = pp.tile([128, 64], F32, tag="avtr", bufs=2, name="ps_av")
                            for ji, j in enumerate(vjs):
                                nc.tensor.matmul(
                                    ps_av[:],
                                    wtt[:, 128 * j : 128 * (j + 1)],
                                    v_tm[:, b - 1 + j, 64 * hh : 64 * (hh + 1)],
                                    start=(ji == 0), stop=(ji == len(vjs) - 1),
                                )
                            nc.vector.tensor_scalar(
                                out=at_tm[:, 64 * hh : 64 * (hh + 1)], in0=ps_av[:],
                                scalar1=rr[:], scalar2=None, op0=OP.mult,
                            )
                        # all_w out (band clip at edges)
                        if b == 0:
                            nc.sync.dma_start(
                                aw_d.ap()[l, 0:128, 0:256], acc[:, 128:384]
                            )
                        elif b == NB - 1:
                            nc.sync.dma_start(
                                aw_d.ap()[l, S - 128 : S, S - 256 : S], acc[:, 0:256]
                            )
                        else:
                            nc.sync.dma_start(
                                aw_d.ap()[l, 128 * b : 128 * (b + 1),
                                          128 * (b - 1) : 128 * (b - 1) + W],
                                acc[:],
                            )
                        # transpose attn block to feature-major
                        for c in range(DC):
                            ps = pp.tile([128, 128], BF16, tag="avtr", bufs=2, name="ps_atr")
                            nc.tensor.transpose(
                                ps[:], at_tm[:, 128 * c : 128 * (c + 1)], idb[:]
                            )
                            nc.vector.tensor_copy(
                                attnF[:, c, 128 * b : 128 * (b + 1)], ps[:]
                            )

                    # ======== out-proj + residual ========
                    for mo in range(DC):
                        wt = wpool.tile([128, DC, 128], BF16, tag="wqk",
                                        bufs=3, name="wo")
                        nc.sync.dma_start(wt[:], wo_d.ap()[l, mo])
                        for no in range(2):
                            sl = slice(512 * no, 512 * (no + 1))
                            ps = pp.tile([128, 512], F32, tag="mm512", bufs=2, name="ps_o")
                            pairs = [(wt[:, kc, :], attnF[:, kc, sl]) for kc in range(DC)]
                            bp = bias_pair("o", l, 128 * mo, 128, 512)
                            if bp:
                                pairs.append(bp)
                            mm_seq(ps[:], pairs)
                            nc.vector.tensor_tensor(
                                out=x[:, mo, sl], in0=x[:, mo, sl], in1=ps[:], op=OP.add
                            )

                    # ======== LN2 + FFN ========
                    layer_norm(x, h2)
                    for no in range(2):
                        sl = slice(512 * no, 512 * (no + 1))
                        for mo in range(FC):
                            wt = wpool.tile([128, DC, 128], BF16, tag="wqk",
                                            bufs=3, name="w1")
                            nc.sync.dma_start(wt[:], w1_d.ap()[l, mo])
                            ps = pp.tile([128, 512], F32, tag="mm512", bufs=2, name="ps_f1")
                            pairs = [(wt[:, kc, :], h2[:, kc, sl]) for kc in range(DC)]
                            bp = bias_pair("f1", l, 128 * mo, 128, 512)
                            if bp:
                                pairs.append(bp)
                            mm_seq(ps[:], pairs)
                            nc.scalar.activation(ff1[:, mo, :], ps[:], AF.Gelu)
                        for mo in range(DC):
                            ps = pp.tile([128, 512], F32, tag="mm512", bufs=2, name="ps_f2")
                            wt = wpool.tile([128, FC, 128], BF16, tag="w2",
                                            bufs=2, name="w2")
                            nc.sync.dma_start(wt[:], w2_d.ap()[l, mo])
                            pairs = [(wt[:, kc, :], ff1[:, kc, :]) for kc in range(FC)]
                            bp = bias_pair("f2", l, 128 * mo, 128, 512)
                            if bp:
                                pairs.append(bp)
                            mm_seq(ps[:], pairs)
                            nc.vector.tensor_tensor(
                                out=x[:, mo, sl], in0=x[:, mo, sl], in1=ps[:], op=OP.add
                            )

                # ======== attention pooling + classifier ========
                for half in range(2):
                    sl = slice(512 * half, 512 * (half + 1))
                    ps = pp.tile([1, 512], F32, tag="mm512", bufs=2, name="ps_pool")
                    for c in range(DC):
                        xb = wk.tile([128, 512], BF16, tag=f"xb{c % 3}",
                                     bufs=2, name="xbp")
                        nc.vector.tensor_copy(xb[:], x[:, c, sl])
                        nc.tensor.matmul(
                            ps[:], wp[:, c : c + 1], xb[:],
                            start=(c == 0), stop=(c == DC - 1),
                        )
                    nc.scalar.activation(s1row[:, sl], ps[:], AF.Copy)
                nc.vector.tensor_tensor(out=s1row[:], in0=s1row[:], in1=pen[:], op=OP.add)
                psum_s = wk.tile([1, 1], F32, tag="pwsum", name="pwsum")
                nc.scalar.activation(s2row[:], s1row[:], AF.Exp, accum_out=psum_s[:])
                pr = wk.tile([1, 1], F32, tag="pr", name="pr")
                nc.vector.reciprocal(pr[:], psum_s[:])
                nc.vector.tensor_scalar(
                    out=s2row[:], in0=s2row[:], scalar1=pr[:], scalar2=None, op0=OP.mult
                )
                nc.gpsimd.partition_broadcast(r_bc[:], s2row[:])
                for c in range(DC):
                    tmp = wk.tile([128, S], F32, tag="lntmp", bufs=2, name="ptmp")
                    nc.vector.tensor_tensor(
                        out=tmp[:], in0=x[:, c, :], in1=r_bc[:], op=OP.mult
                    )
                    nc.vector.tensor_reduce(
                        out=pooled[:, c : c + 1], in_=tmp[:], axis=AX, op=OP.add
                    )
                nc.scalar.activation(pooled[:], pooled[:], AF.Tanh)
                # cls layernorm over 768 (partition reduction over 6 chunks)
                psq = wk.tile([128, DC], F32, tag="psq", name="psq")
                nc.vector.tensor_tensor(
                    out=psq[:], in0=pooled[:], in1=pooled[:], op=OP.mult
                )
                pooledb = wk.tile([128, DC], BF16, tag="pooledb", name="pooledb")
                nc.vector.tensor_copy(pooledb[:], pooled[:])
                psqb = wk.tile([128, DC], BF16, tag="psqb", name="psqb")
                nc.vector.tensor_copy(psqb[:], psq[:])
                st = []
                for which, srct in ((0, pooledb), (1, psqb)):
                    ps = pp.tile([1, 1], F32, tag="avtr", bufs=2, name="ps_cst")
                    for c in range(DC):
                        nc.tensor.matmul(
                            ps[:], ones_col[:], srct[:, c : c + 1],
                            start=(c == 0), stop=(c == DC - 1),
                        )
                    t = wk.tile([1, 1], F32, tag=f"cst{which}", name="cst")
                    nc.scalar.activation(t[:], ps[:], AF.Copy)
                    st.append(t)
                cm = wk.tile([1, 1], F32, tag="cm", name="cm")
                nc.vector.tensor_scalar(
                    out=cm[:], in0=st[0][:], scalar1=1.0 / D, scalar2=None, op0=OP.mult
                )
                cmsq = wk.tile([1, 1], F32, tag="cmsq", name="cmsq")
                nc.vector.tensor_tensor(out=cmsq[:], in0=cm[:], in1=cm[:], op=OP.mult)
                cvar = wk.tile([1, 1], F32, tag="cvar", name="cvar")
                nc.vector.tensor_scalar(
                    out=cvar[:], in0=st[1][:], scalar1=1.0 / D, scalar2=None, op0=OP.mult
                )
                nc.vector.tensor_tensor(
                    out=cvar[:], in0=cvar[:], in1=cmsq[:], op=OP.subtract
                )
                csd = wk.tile([1, 1], F32, tag="csd", name="csd")
                nc.scalar.activation(csd[:], cvar[:], AF.Sqrt, bias=epsc[0:1])
                cr = wk.tile([1, 1], F32, tag="cr", name="cr")
                nc.vector.reciprocal(cr[:], csd[:])
                cm_b = wk.tile([128, 1], F32, tag="cmb", name="cmb")
                nc.gpsimd.partition_broadcast(cm_b[:], cm[:])
                cr_b = wk.tile([128, 1], F32, tag="crb", name="crb")
                nc.gpsimd.partition_broadcast(cr_b[:], cr[:])
                nc.vector.tensor_scalar(
                    out=hc[:], in0=pooled[:], scalar1=cm_b[:], scalar2=cr_b[:],
                    op0=OP.subtract, op1=OP.mult,
                )
                # c1 = tanh(Wc1' @ hc (+bc1))
                for mo in range(DC):
                    ps = pp.tile([128, 1], F32, tag="avtr", bufs=2, name="ps_c1")
                    wt = wpool.tile([128, DC, 128], BF16, tag="wqk",
                                    bufs=3, name="wc1")
                    nc.sync.dma_start(wt[:], wc1_d.ap()[mo])
                    pairs = [(wt[:, kc, :], hc[:, kc : kc + 1]) for kc in range(DC)]
                    if "c1" in bias_d:
                        pairs.append(
                            (bc1[:, 128 * mo : 128 * (mo + 1)], ones_row[:, :1])
                        )
                    mm_seq(ps[:], pairs)
                    nc.scalar.activation(c1[:, mo : mo + 1], ps[:], AF.Tanh)
                # out = Wc2 @ c1 (+bc2)
                ps = pp.tile([2, 1], F32, tag="avtr", bufs=2, name="ps_c2")
                for kc in range(DC):
                    nc.tensor.matmul(
                        ps[:], wc2[:, kc, :], c1[:, kc : kc + 1],
                        start=(kc == 0), stop=(kc == DC - 1),
                    )
                ores = wk.tile([2, 1], F32, tag="ores", name="ores")
                if "c2" in bias_d:
                    nc.vector.tensor_scalar(
                        out=ores[:], in0=ps[:], scalar1=bc2[:], scalar2=None, op0=OP.add
                    )
                else:
                    nc.vector.tensor_copy(ores[:], ps[:])
                nc.sync.dma_start(out_d.ap().rearrange("(a b) -> a b", b=1), ores[:])

    nc.finalize()
    return nc


def _host_prep(tokens, mask, emb, in_w, in_b, out_w, out_b, ln1_s, ln1_b, ln2_s,
               ln2_b, ff_w1, ff_b1, ff_w2, ff_b2, pool_w, pool_b, cls_ln_s,
               cls_ln_b, cls_w1, cls_b1, cls_w2, cls_b2):
    f32 = np.float32
    tokens = np.asarray(tokens).astype(np.int32)
    mask = np.asarray(mask).astype(np.int32)
    emb = np.ascontiguousarray(np.asarray(emb, f32))

    # positional encoding (match reference f32 math)
    pos = np.arange(S, dtype=f32)[:, None]
    div = np.exp(np.arange(0, D, 2, dtype=f32) * f32(-np.log(10000.0) / D))
    pe = np.zeros((S, D), f32)
    pe[:, 0::2] = np.sin(pos * div)
    pe[:, 1::2] = np.cos(pos * div)
    pe_t = np.ascontiguousarray(pe.T.reshape(DC, 128, S))

    def blocked(wT, nm):  # [din, dout] -> [nm, 128, DC, 128]: per-mo contiguous
        return np.ascontiguousarray(
            wT.reshape(DC, 128, nm, 128).transpose(2, 1, 0, 3).astype(nbf)
        )

    wqk_t = np.zeros((L, 12, 128, DC, 128), nbf)
    wv_t = np.zeros((L, 2, 128, DC, 384), nbf)
    wo_t = np.zeros((L, DC, 128, DC, 128), nbf)
    w1_t = np.zeros((L, FC, 128, DC, 128), nbf)
    w2_t = np.zeros((L, DC, 128, FC, 128), nbf)
    b_qk = np.zeros((L, 1, 1536), f32)
    b_v = np.zeros((L, 1, D), f32)
    b_o = np.zeros((L, 1, D), f32)
    b_f1 = np.zeros((L, 1, F), f32)
    b_f2 = np.zeros((L, 1, D), f32)
    for l in range(L):
        Wi = np.asarray(in_w[l], f32)
        bi = np.asarray(in_b[l], f32) + Wi @ np.asarray(ln1_b[l], f32)
        Wi = Wi * np.asarray(ln1_s[l], f32)[None, :]
        qs = f32(1.0 / np.sqrt(D // H))
        Wi = np.concatenate([Wi[:D] * qs, Wi[D:]], axis=0)
        bi = np.concatenate([bi[:D] * qs, bi[D:]], axis=0)
        wqk_t[l] = blocked(Wi[:1536].T, 12)
        wv_t[l] = np.ascontiguousarray(
            Wi[1536:].T.reshape(DC, 128, 2, 384).transpose(2, 1, 0, 3)
        ).astype(nbf)
        b_qk[l, 0] = bi[:1536]
        b_v[l, 0] = bi[1536:]
        wo_t[l] = blocked(np.asarray(out_w[l], f32).T, DC)
        b_o[l, 0] = np.asarray(out_b[l], f32)
        W1 = np.asarray(ff_w1[l], f32)
        b1 = np.asarray(ff_b1[l], f32) + W1 @ np.asarray(ln2_b[l], f32)
        W1 = W1 * np.asarray(ln2_s[l], f32)[None, :]
        w1_t[l] = blocked(W1.T, FC)
        b_f1[l, 0] = b1
        w2_t[l] = np.ascontiguousarray(
            np.asarray(ff_w2[l], f32).T.reshape(FC, 128, DC, 128)
            .transpose(2, 1, 0, 3).astype(nbf)
        )
        b_f2[l, 0] = np.asarray(ff_b2[l], f32)

    Wc1 = np.asarray(cls_w1, f32)
    bc1 = np.asarray(cls_b1, f32) + Wc1 @ np.asarray(cls_ln_b, f32)
    Wc1 = Wc1 * np.asarray(cls_ln_s, f32)[None, :]
    wc1_t = blocked(Wc1.T, DC)
    wc2_t = np.ascontiguousarray(
        np.asarray(cls_w2, f32).T.reshape(DC, 128, 2).astype(nbf)
    )
    b_c2 = np.asarray(cls_b2, f32).reshape(2, 1)
    wp_t = np.ascontiguousarray(
        np.asarray(pool_w, f32).reshape(D).reshape(DC, 128).T.astype(nbf)
    )  # [128, DC]

    # masks per core (band + key padding), token-major and transposed
    li = np.arange(128)[:, None]
    wcol = np.arange(W)[None, :]
    band = (wcol >= li) & (wcol <= li + 256)
    mask_tm = np.zeros((B, NB, 128, W), nbf)
    mask_tt = np.zeros((B, NB, 3, 128, 128), nbf)
    for bb in range(B):
        kpm = mask[bb] != 0
        for qb in range(NB):
            k_real = 128 * (qb - 1) + wcol
            in_range = (k_real >= 0) & (k_real < S)
            valid = band & in_range & kpm[np.clip(k_real, 0, S - 1)]
            m = np.where(valid, f32(0.0), f32(-30000.0))
            mask_tm[bb, qb] = m.astype(nbf)
            mask_tt[bb, qb] = np.ascontiguousarray(
                m.reshape(128, 3, 128).transpose(1, 2, 0)
            ).astype(nbf)

    penalty = (
        np.where(mask == 0, f32(-1e30), f32(0.0))
        + np.asarray(pool_b, f32).reshape(1)[0]
    ).astype(f32).reshape(B, 1, S)

    shared = dict(
        emb=emb, pe_t=pe_t,
        ident_f32=np.eye(128, dtype=f32),
        ident_bf16=np.eye(128, dtype=f32).astype(nbf),
        wqk_t=wqk_t, wv_t=wv_t, wo_t=wo_t, w1_t=w1_t, w2_t=w2_t,
        wc1_t=wc1_t, wc2_t=wc2_t, wp_t=wp_t,
    )
    maybe_bias = dict(
        qk=b_qk.astype(nbf), v=b_v.astype(nbf), o=b_o.astype(nbf),
        f1=b_f1.astype(nbf), f2=b_f2.astype(nbf),
        c1=bc1.astype(nbf).reshape(1, D), c2=b_c2,
    )
    apply_biases = {
        k: bool(np.any(np.asarray(v, f32) != 0)) for k, v in maybe_bias.items()
    }
    bias_inputs = {
        "b_" + k: v for k, v in maybe_bias.items() if apply_biases[k]
    }
    per_core = [
        dict(tokens=tokens[bb], penalty=penalty[bb],
             mask_tm=mask_tm[bb], mask_tt=mask_tt[bb])
        for bb in range(B)
    ]
    return shared, bias_inputs, apply_biases, per_core


def kernel(**inputs):
    global LAST_RESULTS
    shared, bias_inputs, apply_biases, per_core = _host_prep(**inputs)

    key = tuple(sorted(apply_biases.items()))
    if key not in _CACHE:
        _CACHE[key] = _build_nc(apply_biases)
    nc = _CACHE[key]

    in_maps = []
    for bb in range(B):
        m = dict(shared)
        m.update(bias_inputs)
        m.update(per_core[bb])
        in_maps.append(m)

    res = run_bass_kernel_spmd(nc, in_maps, list(range(B)), trace=False)
    LAST_RESULTS = res

    out = np.stack([np.asarray(res.results[bb]["cls_out"]) for bb in range(B)])
    all_w = np.stack([np.asarray(res.results[bb]["all_w"]) for bb in range(B)])
    all_w = np.ascontiguousarray(all_w.transpose(1, 0, 2, 3))  # [L, B, S, S]
    return out.astype(np.float32), all_w.astype(np.float32)


def time_kernel(reps=20, **inputs):
    """Repeat-execute the compiled NEFF (non-donated buffers, inputs resident
    on device) and return per-call wall seconds. Mirrors
    bass2jax.run_bass_via_pjrt's sharded setup minus donation."""
    import time
    import jax
    import concourse.mybir as mybir_
    from concourse import bass2jax
    from jax.experimental.shard_map import shard_map
    from jax.sharding import Mesh, PartitionSpec

    shared, bias_inputs, apply_biases, per_core = _host_prep(**inputs)
    key = tuple(sorted(apply_biases.items()))
    if key not in _CACHE:
        _CACHE[key] = _build_nc(apply_biases)
    nc = _CACHE[key]
    in_maps = []
    for bb in range(B):
        m = dict(shared)
        m.update(bias_inputs)
        m.update(per_core[bb])
        in_maps.append(m)

    bass2jax.install_neuronx_cc_hook()
    n_cores = B
    in_names, out_names, out_avals, zero_outs = [], [], [], []
    for alloc in nc.m.functions[0].allocations:
        if not isinstance(alloc, mybir_.MemoryLocationSet):
            continue
        name = alloc.memorylocations[0].name
        if alloc.kind == "ExternalInput":
            in_names.append(name)
        elif alloc.kind == "ExternalOutput":
            out_names.append(name)
            shape = tuple(alloc.tensor_shape)
            dtype = mybir_.dt.np(alloc.dtype)
            out_avals.append(jax.core.ShapedArray(shape, dtype))
            zero_outs.append(np.zeros(shape, dtype))
    partition_name = (
        nc.partition_id_tensor.name if nc.partition_id_tensor else None
    )
    if partition_name in in_names:
        in_names.remove(partition_name)
    n_params = len(in_names)
    all_in_names = in_names + out_names
    if partition_name is not None:
        all_in_names.append(partition_name)

    def _body(*args):
        operands = list(args)
        if partition_name is not None:
            operands.append(bass2jax.partition_id_tensor())
        outs = bass2jax._bass_exec_p.bind(
            *operands,
            out_avals=tuple(out_avals),
            in_names=tuple(all_in_names),
            out_names=tuple(out_names),
            lowering_input_output_aliases=(),
            sim_require_finite=True,
            sim_require_nnan=True,
            nc=nc,
        )
        return tuple(outs)

    devices = jax.devices()[:n_cores]
    mesh = Mesh(np.asarray(devices), ("core",))
    n_outs = len(out_names)
    sharded = jax.jit(
        shard_map(
            _body, mesh=mesh,
            in_specs=(PartitionSpec("core"),) * (n_params + n_outs),
            out_specs=(PartitionSpec("core"),) * n_outs,
            check_rep=False,
        ),
        keep_unused=True,
    )
    concat_in = [
        np.concatenate([np.asarray(in_maps[c][nm]) for c in range(n_cores)], axis=0)
        for nm in in_names
    ]
    concat_zeros = [
        np.zeros((n_cores * z.shape[0], *z.shape[1:]), z.dtype) for z in zero_outs
    ]
    from jax.sharding import NamedSharding
    args = [
        jax.device_put(a, NamedSharding(mesh, PartitionSpec("core")))
        for a in concat_in + concat_zeros
    ]
    # warm-up (compile + first exec)
    r = sharded(*args)
    jax.block_until_ready(r)
    times = []
    for _ in range(3):
        t0 = time.perf_counter()
        for _ in range(reps):
            r = sharded(*args)
        jax.block_until_ready(r)
        times.append((time.perf_counter() - t0) / reps)
    return times


# revision 19
# speedup vs baseline: 62.1566x; 62.1566x over previous
"""Trainium2 Bass kernel for nn_BERT_89017492177566 (sparse sliding-window BERT).

Data-parallel over batch: 8 sequences -> 8 NeuronCores, one sequence each.
Activations kept feature-major (FM: features on partitions, tokens on free axis)
in SBUF; all big matmuls bf16 with f32 PSUM accumulation. LayerNorm scale/bias
and the 1/sqrt(hd) query scale are folded into the weights on the host.
Banded attention (|q-k| <= 128) computed per 128-query block against a 384-wide
key window; softmax weights are produced in both orientations (token-major for
row sums + the all_w output, transposed for the AV matmul).
"""

import os
import numpy as np
import ml_dtypes

import concourse.bass as bass
import concourse.bacc as bacc
import concourse.mybir as mybir
import concourse.tile as tile
from concourse.bass_utils import run_bass_kernel_spmd

BF16 = mybir.dt.bfloat16
F32 = mybir.dt.float32
F32R = mybir.dt.float32r
I32 = mybir.dt.int32

H = 12
EPS = 1e-5
S = 1024
D = 768
F = 3072
L = 4
B = 8
NB = S // 128           # 8 token blocks
DC = D // 128           # 6 feature chunks
FC = F // 128           # 24 ff chunks
W = 3 * 128             # key window width

nbf = ml_dtypes.bfloat16

_CACHE = {}
LAST_RESULTS = None


def _build_nc(apply_biases):
    """Emit the full Bass/Tile program. apply_biases: dict of bools per bias group."""
    nc = bacc.Bacc("TRN2", target_bir_lowering=False, debug=False, num_devices=8)

    # ---------------- DRAM tensors ----------------
    emb_d = nc.dram_tensor("emb", [32000, D], F32, kind="ExternalInput")
    tok_d = nc.dram_tensor("tokens", [S], I32, kind="ExternalInput")
    pen_d = nc.dram_tensor("penalty", [1, S], F32, kind="ExternalInput")
    pet_d = nc.dram_tensor("pe_t", [DC, 128, S], F32, kind="ExternalInput")
    mtm_d = nc.dram_tensor("mask_tm", [NB, 128, W], BF16, kind="ExternalInput")
    mtt_d = nc.dram_tensor("mask_tt", [NB, 3, 128, 128], BF16, kind="ExternalInput")
    idf_d = nc.dram_tensor("ident_f32", [128, 128], F32, kind="ExternalInput")
    idb_d = nc.dram_tensor("ident_bf16", [128, 128], BF16, kind="ExternalInput")

    wqk_d = nc.dram_tensor("wqk_t", [L, 12, 128, DC, 128], BF16, kind="ExternalInput")
    wv_d = nc.dram_tensor("wv_t", [L, 2, 128, DC, 384], BF16, kind="ExternalInput")
    wo_d = nc.dram_tensor("wo_t", [L, DC, 128, DC, 128], BF16, kind="ExternalInput")
    w1_d = nc.dram_tensor("w1_t", [L, FC, 128, DC, 128], BF16, kind="ExternalInput")
    w2_d = nc.dram_tensor("w2_t", [L, DC, 128, FC, 128], BF16, kind="ExternalInput")
    wc1_d = nc.dram_tensor("wc1_t", [DC, 128, DC, 128], BF16, kind="ExternalInput")
    wc2_d = nc.dram_tensor("wc2_t", [DC, 128, 2], BF16, kind="ExternalInput")
    wp_d = nc.dram_tensor("wp_t", [128, DC], BF16, kind="ExternalInput")

    bias_d = {}
    for key, shp, dt_ in (
        ("qk", [L, 1, 1536], BF16), ("v", [L, 1, D], BF16), ("o", [L, 1, D], BF16),
        ("f1", [L, 1, F], BF16), ("f2", [L, 1, D], BF16),
        ("c1", [1, D], BF16), ("c2", [2, 1], F32),
    ):
        if apply_biases[key]:
            bias_d[key] = nc.dram_tensor("b_" + key, shp, dt_, kind="ExternalInput")

    out_d = nc.dram_tensor("cls_out", [2], F32, kind="ExternalOutput")
    aw_d = nc.dram_tensor("all_w", [L, S, S], F32, kind="ExternalOutput")

    AX = mybir.AxisListType.X
    OP = mybir.AluOpType
    AF = mybir.ActivationFunctionType

    def mm_seq(ps, pairs):
        n = len(pairs)
        for i, (lt, rh) in enumerate(pairs):
            nc.tensor.matmul(ps, lt, rh, start=(i == 0), stop=(i == n - 1))

    with tile.TileContext(nc) as tc:
        with (
            tc.tile_pool(name="const", bufs=1) as cpool,
            tc.tile_pool(name="state", bufs=1) as spool,
        ):
            # ---------------- constants ----------------
            idf = cpool.tile([128, 128], F32)
            nc.sync.dma_start(idf[:], idf_d.ap())
            idb = cpool.tile([128, 128], BF16)
            nc.sync.dma_start(idb[:], idb_d.ap())
            mtm = cpool.tile([128, NB, W], BF16)
            nc.sync.dma_start(mtm[:], mtm_d.ap().rearrange("b p w -> p b w"))
            mtt = cpool.tile([128, NB, 3, 128], BF16)
            nc.sync.dma_start(mtt[:], mtt_d.ap().rearrange("b j p q -> p b j q"))
            idx = cpool.tile([128, NB], I32)
            nc.sync.dma_start(idx[:], tok_d.ap().rearrange("(b p) -> p b", p=128))
            pen = cpool.tile([1, S], F32)
            nc.sync.dma_start(pen[:], pen_d.ap())
            wp = cpool.tile([128, DC], BF16)
            nc.sync.dma_start(wp[:], wp_d.ap())
            wc2 = cpool.tile([128, DC, 2], BF16)
            nc.sync.dma_start(wc2[:], wc2_d.ap().rearrange("c p t -> p c t"))
            ones_col = cpool.tile([128, 1], BF16)
            nc.gpsimd.memset(ones_col[:], 1.0)
            ones_row = cpool.tile([1, 512], BF16)
            nc.gpsimd.memset(ones_row[:], 1.0)
            epsc = cpool.tile([128, 1], F32)
            nc.gpsimd.memset(epsc[:], EPS)
            btiles = {}
            for key in ("qk", "v", "o", "f1", "f2"):
                if key in bias_d:
                    n_ = bias_d[key].shape[2]
                    t = cpool.tile([1, L * n_], BF16, name=f"bt_{key}")
                    nc.sync.dma_start(
                        t[:], bias_d[key].ap().rearrange("l o n -> o (l n)")
                    )
                    btiles[key] = (t, n_)
            if "c1" in bias_d:
                bc1 = cpool.tile([1, D], BF16)
                nc.sync.dma_start(bc1[:], bias_d["c1"].ap())
            if "c2" in bias_d:
                bc2 = cpool.tile([2, 1], F32)
                nc.sync.dma_start(bc2[:], bias_d["c2"].ap())

            def bias_pair(key, l, off, width, nrep, fm=True):
                """Rank-1 bias matmul operands, or None. fm: bias on partitions."""
                if key not in btiles:
                    return None
                row, n_ = btiles[key]
                base = l * n_
                sl = row[:, base + off : base + off + width]
                if fm:
                    return (sl, ones_row[:, :nrep])
                return (ones_row[:, :nrep], sl)

            # ---------------- persistent state ----------------
            x = spool.tile([128, DC, S], F32)           # residual, feature-major
            h = spool.tile([128, DC, S], BF16)          # LN1/LN2 output (reused)
            h2 = h
            qT = spool.tile([128, DC, S], BF16)
            kTp = spool.tile([128, DC, S + 256], BF16)  # padded keys
            v_tm = spool.tile([128, NB, D], BF16)       # token-major values
            attnF = spool.tile([128, DC, S], BF16)      # attention out, feature-major
            ff1 = spool.tile([128, FC, 512], BF16)      # one token-half at a time
            s1row = spool.tile([1, S], F32)   # stat sums (partition 0)
            s2row = spool.tile([1, S], F32)
            bcA = spool.tile([128, S], F32)   # sum -> mean -> mean*rstd
            bcB = spool.tile([128, S], F32)   # sumsq -> var -> sd
            r_bc = spool.tile([128, S], F32)  # scratch -> rstd (broadcast)
            pooled = spool.tile([128, DC], F32)
            hc = spool.tile([128, DC], BF16)
            c1 = spool.tile([128, DC], BF16)

            # zero the key pads once; never written again
            nc.gpsimd.memset(kTp[:, :, 0:128], 0.0)
            nc.gpsimd.memset(kTp[:, :, S + 128 : S + 256], 0.0)

            # ---------------- embedding + positional ----------------
            with (
                tc.tile_pool(name="embed", bufs=2) as epool,
                tc.tile_pool(name="embps", bufs=2, space="PSUM") as eps,
            ):
                # absorb identity-DMA waits into PE's clock (transpose-mode
                # LDWEIGHTS supports only one sync wait slot)
                ps_ab = eps.tile([128, 128], F32, tag="tr", name="ps_ab")
                nc.tensor.transpose(ps_ab[:], idf[:], idf[:])
                ps_abb = eps.tile([128, 128], BF16, tag="trb", name="ps_abb")
                nc.tensor.transpose(ps_abb[:], idb[:], idb[:])
                pe_c = []
                for c in range(DC):
                    t = epool.tile([128, S], F32, tag=f"pe{c}", bufs=1, name=f"pe{c}")
                    nc.sync.dma_start(t[:], pet_d.ap()[c])
                    pe_c.append(t)
                for tb in range(NB):
                    x0 = epool.tile([128, D], F32, tag="x0", name="x0")
                    nc.gpsimd.indirect_dma_start(
                        out=x0[:],
                        out_offset=None,
                        in_=emb_d.ap(),
                        in_offset=bass.IndirectOffsetOnAxis(
                            ap=idx[:, tb : tb + 1], axis=0
                        ),
                    )
                    # bounce through DVE so every transpose dep is in the DVE
                    # sem domain (single wait on the PE side)
                    x0b = epool.tile([128, D], F32, tag="x0b", name="x0b")
                    nc.vector.tensor_copy(x0b[:], x0[:])
                    for c in range(DC):
                        ps = eps.tile([128, 128], F32, tag="tr", name="ps_tr")
                        nc.tensor.transpose(ps[:], x0b[:, 128 * c : 128 * (c + 1)], idf[:])
                        nc.vector.tensor_tensor(
                            out=x[:, c, 128 * tb : 128 * (tb + 1)],
                            in0=ps[:],
                            in1=pe_c[c][:, 128 * tb : 128 * (tb + 1)],
                            op=OP.add,
                        )

            # ---------------- main work pools ----------------
            with (
                tc.tile_pool(name="wts", bufs=1) as wpool,
                tc.tile_pool(name="work", bufs=1) as wk,
                tc.tile_pool(name="atw", bufs=3) as aw,
                tc.tile_pool(name="ps", bufs=1, space="PSUM") as pp,
            ):

                def layer_norm(src, dst):
                    """dst(bf16) = (src - mean) * rstd, feature-major."""
                    for half in range(2):
                        sl = slice(512 * half, 512 * (half + 1))
                        # sums of x (bf16 casts just-in-time)
                        ps = pp.tile([1, 512], F32, tag="mm512", bufs=2, name="ps_st")
                        xbs = []
                        for c in range(DC):
                            xb = wk.tile([128, 512], BF16, tag=f"xb{c % 3}",
                                         bufs=2, name="xb")
                            nc.vector.tensor_copy(xb[:], src[:, c, sl])
                            xbs.append(xb)
                            nc.tensor.matmul(
                                ps[:], ones_col[:], xb[:],
                                start=(c == 0), stop=(c == DC - 1),
                            )
                        nc.scalar.activation(s1row[:, sl], ps[:], AF.Copy)
                        # sums of x^2
                        ps2 = pp.tile([1, 512], F32, tag="mm512", bufs=2, name="ps_st2")
                        for c in range(DC):
                            xsq = wk.tile([128, 512], BF16, tag="xsq", bufs=2, name="xsq")
                            nc.vector.tensor_tensor(
                                out=xsq[:], in0=src[:, c, sl], in1=src[:, c, sl],
                                op=OP.mult,
                            )
                            nc.tensor.matmul(
                                ps2[:], ones_col[:], xsq[:],
                                start=(c == 0), stop=(c == DC - 1),
                            )
                        nc.scalar.activation(s2row[:, sl], ps2[:], AF.Copy)
                    # broadcast sums to all partitions, then full-width math:
                    # bcA: mean -> mean*rstd ; bcB: var -> sd ; r_bc: rstd
                    nc.gpsimd.partition_broadcast(bcA[:], s1row[:])
                    nc.gpsimd.partition_broadcast(bcB[:], s2row[:])
                    nc.vector.tensor_scalar(
                        out=bcA[:], in0=bcA[:], scalar1=1.0 / D, scalar2=None,
                        op0=OP.mult,
                    )
                    nc.vector.tensor_tensor(
                        out=r_bc[:], in0=bcA[:], in1=bcA[:], op=OP.mult
                    )
                    nc.vector.tensor_scalar(
                        out=bcB[:], in0=bcB[:], scalar1=1.0 / D, scalar2=None,
                        op0=OP.mult,
                    )
                    nc.vector.tensor_tensor(
                        out=bcB[:], in0=bcB[:], in1=r_bc[:], op=OP.subtract
                    )
                    nc.scalar.activation(bcB[:], bcB[:], AF.Sqrt, bias=epsc[:])
                    nc.vector.reciprocal(r_bc[:], bcB[:])
                    nc.vector.tensor_tensor(
                        out=bcA[:], in0=bcA[:], in1=r_bc[:], op=OP.mult
                    )
                    for c in range(DC):
                        tmp = wk.tile([128, S], F32, tag="lntmp", bufs=2, name="lntmp")
                        nc.vector.tensor_tensor(
                            out=tmp[:], in0=src[:, c, :], in1=r_bc[:], op=OP.mult
                        )
                        nc.vector.tensor_tensor(
                            out=dst[:, c, :], in0=tmp[:], in1=bcA[:], op=OP.subtract
                        )

                for l in range(L):
                    # ======== LN1 ========
                    layer_norm(x, h)

                    # ======== QKV: q,k feature-major ========
                    for mo in range(12):
                        wt = wpool.tile([128, DC, 128], BF16, tag="wqk",
                                        bufs=3, name="wqk")
                        nc.sync.dma_start(wt[:], wqk_d.ap()[l, mo])
                        for no in range(2):
                            sl = slice(512 * no, 512 * (no + 1))
                            ps = pp.tile([128, 512], F32, tag="mm512", bufs=2, name="ps_qk")
                            pairs = [(wt[:, kc, :], h[:, kc, sl]) for kc in range(DC)]
                            bp = bias_pair("qk", l, 128 * mo, 128, 512)
                            if bp:
                                pairs.append(bp)
                            mm_seq(ps[:], pairs)
                            if mo < 6:
                                nc.scalar.activation(qT[:, mo, sl], ps[:], AF.Copy)
                            else:
                                nc.scalar.activation(
                                    kTp[:, mo - 6,
                                        slice(128 + 512 * no, 128 + 512 * (no + 1))],
                                    ps[:], AF.Copy,
                                )

                    # ======== V token-major ========
                    for no in range(2):
                        wv = wpool.tile([128, DC, 384], BF16, tag="wv",
                                        bufs=1, name="wv")
                        nc.sync.dma_start(wv[:], wv_d.ap()[l, no])
                        for tb in range(NB):
                            ps = pp.tile([128, 384], F32, tag="s_tm", bufs=2, name="ps_v")
                            pairs = [
                                (h[:, kc, 128 * tb : 128 * (tb + 1)], wv[:, kc, :])
                                for kc in range(DC)
                            ]
                            bp = bias_pair("v", l, 384 * no, 384, 128, fm=False)
                            if bp:
                                pairs.append(bp)
                            mm_seq(ps[:], pairs)
                            nc.scalar.activation(
                                v_tm[:, tb, 384 * no : 384 * (no + 1)], ps[:], AF.Copy
                            )

                    # ======== banded attention ========
                    for b in range(NB):
                        acc = wk.tile([128, W], F32, tag=f"acc{b & 1}", name="acc")
                        at_tm = wk.tile([128, D], BF16, tag=f"attm{b & 1}", name="at_tm")
                        vjs = [j for j in range(3) if 0 <= b - 1 + j <= 7]
                        for hh in range(H):
                            c_h, o_h = (64 * hh) // 128, (64 * hh) % 128
                            q_h = qT[o_h : o_h + 64, c_h, 128 * b : 128 * (b + 1)]
                            ps_s = pp.tile([128, W], F32, tag="s_tm", bufs=2, name="ps_s")
                            nc.tensor.matmul(
                                ps_s[:], q_h,
                                kTp[o_h : o_h + 64, c_h, 128 * b : 128 * b + W],
                                start=True, stop=False,
                            )
                            nc.tensor.matmul(
                                ps_s[:], idb[:], mtm[:, b, :],
                                start=False, stop=True,
                            )
                            ps_t = pp.tile([128, W], F32, tag="sT", bufs=2, name="ps_t")
                            for j in range(3):
                                nc.tensor.matmul(
                                    ps_t[:, 128 * j : 128 * (j + 1)],
                                    kTp[o_h : o_h + 64, c_h,
                                        128 * (b + j) : 128 * (b + j + 1)],
                                    q_h,
                                    start=True, stop=False,
                                )
                                nc.tensor.matmul(
                                    ps_t[:, 128 * j : 128 * (j + 1)],
                                    idb[:], mtt[:, b, j, :],
                                    start=False, stop=True,
                                )
                            ssum = aw.tile([128, 1], F32, tag="ssum", name="ssum")
                            wtm = aw.tile([128, W], BF16, tag="wtm", name="wtm")
                            nc.scalar.activation(
                                wtm[:], ps_s[:], AF.Exp, accum_out=ssum[:]
                            )
                            wtt = aw.tile([128, W], BF16, tag="wtt", name="wtt")
                            nc.scalar.activation(wtt[:], ps_t[:], AF.Exp)
                            rr = aw.tile([128, 1], F32, tag="rr", name="rr")
                            nc.vector.reciprocal(rr[:], ssum[:])
                            # all_w accumulation: acc (+)= wtm * rr / H
                            if hh == 0:
                                nc.vector.tensor_scalar(
                                    out=acc[:], in0=wtm[:], scalar1=rr[:],
                                    scalar2=1.0 / H, op0=OP.mult, op1=OP.mult,
                                )
                            else:
                                tmp = aw.tile([128, W], F32, tag="awtmp", name="awtmp")
                                nc.vector.tensor_scalar(
                                    out=tmp[:], in0=wtm[:], scalar1=rr[:],
                                    scalar2=1.0 / H, op0=OP.mult, op1=OP.mult,
                                )
                                nc.gpsimd.tensor_tensor(
                                    out=acc[:], in0=acc[:], in1=tmp[:], op=OP.add
                                )
                            # AV -> token-major attn head, normalized at evac
                            ps_av = pp.tile([128, 64], F32, tag="avtr", bufs=2, name="ps_av")
                            for ji, j in enumerate(vjs):
                                nc.tensor.matmul(
                                    ps_av[:],
                                    wtt[:, 128 * j : 128 * (j + 1)],
                                    v_tm[:, b - 1 + j, 64 * hh : 64 * (hh + 1)],
                                    start=(ji == 0), stop=(ji == len(vjs) - 1),
                                )
                            nc.vector.tensor_scalar(
                                out=at_tm[:, 64 * hh : 64 * (hh + 1)], in0=ps_av[:],
                                scalar1=rr[:], scalar2=None, op0=OP.mult,
                            )
                        # all_w out (band clip at edges)
                        if b == 0:
                            nc.sync.dma_start(
                                aw_d.ap()[l, 0:128, 0:256], acc[:, 128:384]
                            )
                        elif b == NB - 1:
                            nc.sync.dma_start(
                                aw_d.ap()[l, S - 128 : S, S - 256 : S], acc[:, 0:256]
                            )
                        else:
                            nc.sync.dma_start(
                                aw_d.ap()[l, 128 * b : 128 * (b + 1),
                                          128 * (b - 1) : 128 * (b - 1) + W],
                                acc[:],
                            )
                        # transpose attn block to feature-major
                        for c in range(DC):
                            ps = pp.tile([128, 128], BF16, tag="avtr", bufs=2, name="ps_atr")
                            nc.tensor.transpose(
                                ps[:], at_tm[:, 128 * c : 128 * (c + 1)], idb[:]
                            )
                            nc.vector.tensor_copy(
                                attnF[:, c, 128 * b : 128 * (b + 1)], ps[:]
                            )

                    # ======== out-proj + residual ========
                    for mo in range(DC):
                        wt = wpool.tile([128, DC, 128], BF16, tag="wqk",
                                        bufs=3, name="wo")
                        nc.sync.dma_start(wt[:], wo_d.ap()[l, mo])
                        for no in range(2):
                            sl = slice(512 * no, 512 * (no + 1))
                            ps = pp.tile([128, 512], F32, tag="mm512", bufs=2, name="ps_o")
                            pairs = [(wt[:, kc, :], attnF[:, kc, sl]) for kc in range(DC)]
                            bp = bias_pair("o", l, 128 * mo, 128, 512)
                            if bp:
                                pairs.append(bp)
                            mm_seq(ps[:], pairs)
                            nc.vector.tensor_tensor(
                                out=x[:, mo, sl], in0=x[:, mo, sl], in1=ps[:], op=OP.add
                            )

                    # ======== LN2 + FFN ========
                    layer_norm(x, h2)
                    for no in range(2):
                        sl = slice(512 * no, 512 * (no + 1))
                        for mo in range(FC):
                            wt = wpool.tile([128, DC, 128], BF16, tag="wqk",
                                            bufs=3, name="w1")
                            nc.sync.dma_start(wt[:], w1_d.ap()[l, mo])
                            ps = pp.tile([128, 512], F32, tag="mm512", bufs=2, name="ps_f1")
                            pairs = [(wt[:, kc, :], h2[:, kc, sl]) for kc in range(DC)]
                            bp = bias_pair("f1", l, 128 * mo, 128, 512)
                            if bp:
                                pairs.append(bp)
                            mm_seq(ps[:], pairs)
                            nc.scalar.activation(ff1[:, mo, :], ps[:], AF.Gelu)
                        for mo in range(DC):
                            ps = pp.tile([128, 512], F32, tag="mm512", bufs=2, name="ps_f2")
                            wt = wpool.tile([128, FC, 128], BF16, tag="w2",
                                            bufs=2, name="w2")
                            nc.sync.dma_start(wt[:], w2_d.ap()[l, mo])
                            pairs = [(wt[:, kc, :], ff1[:, kc, :]) for kc in range(FC)]
                            bp = bias_pair("f2", l, 128 * mo, 128, 512)
                            if bp:
                                pairs.append(bp)
                            mm_seq(ps[:], pairs)
                            nc.vector.tensor_tensor(
                                out=x[:, mo, sl], in0=x[:, mo, sl], in1=ps[:], op=OP.add
                            )

                # ======== attention pooling + classifier ========
                for half in range(2):
                    sl = slice(512 * half, 512 * (half + 1))
                    ps = pp.tile([1, 512], F32, tag="mm512", bufs=2, name="ps_pool")
                    for c in range(DC):
                        xb = wk.tile([128, 512], BF16, tag=f"xb{c % 3}",
                                     bufs=2, name="xbp")
                        nc.vector.tensor_copy(xb[:], x[:, c, sl])
                        nc.tensor.matmul(
                            ps[:], wp[:, c : c + 1], xb[:],
                            start=(c == 0), stop=(c == DC - 1),
                        )
                    nc.scalar.activation(s1row[:, sl], ps[:], AF.Copy)
                nc.vector.tensor_tensor(out=s1row[:], in0=s1row[:], in1=pen[:], op=OP.add)
                psum_s = wk.tile([1, 1], F32, tag="pwsum", name="pwsum")
                nc.scalar.activation(s2row[:], s1row[:], AF.Exp, accum_out=psum_s[:])
                pr = wk.tile([1, 1], F32, tag="pr", name="pr")
                nc.vector.reciprocal(pr[:], psum_s[:])
                nc.vector.tensor_scalar(
                    out=s2row[:], in0=s2row[:], scalar1=pr[:], scalar2=None, op0=OP.mult
                )
                nc.gpsimd.partition_broadcast(r_bc[:], s2row[:])
                for c in range(DC):
                    tmp = wk.tile([128, S], F32, tag="lntmp", bufs=2, name="ptmp")
                    nc.vector.tensor_tensor(
                        out=tmp[:], in0=x[:, c, :], in1=r_bc[:], op=OP.mult
                    )
                    nc.vector.tensor_reduce(
                        out=pooled[:, c : c + 1], in_=tmp[:], axis=AX, op=OP.add
                    )
                nc.scalar.activation(pooled[:], pooled[:], AF.Tanh)
                # cls layernorm over 768 (partition reduction over 6 chunks)
                psq = wk.tile([128, DC], F32, tag="psq", name="psq")
                nc.vector.tensor_tensor(
                    out=psq[:], in0=pooled[:], in1=pooled[:], op=OP.mult
                )
                pooledb = wk.tile([128, DC], BF16, tag="pooledb", name="pooledb")
                nc.vector.tensor_copy(pooledb[:], pooled[:])
                psqb = wk.tile([128, DC], BF16, tag="psqb", name="psqb")
                nc.vector.tensor_copy(psqb[:], psq[:])
                st = []
                for which, srct in ((0, pooledb), (1, psqb)):
                    ps = pp.tile([1, 1], F32, tag="avtr", bufs=2, name="ps_cst")
                    for c in range(DC):
                        nc.tensor.matmul(
                            ps[:], ones_col[:], srct[:, c : c + 1],
                            start=(c == 0), stop=(c == DC - 1),
                        )
                    t = wk.tile([1, 1], F32, tag=f"cst{which}", name="cst")
                    nc.scalar.activation(t[:], ps[:], AF.Copy)
                    st.append(t)
                cm = wk.tile([1, 1], F32, tag="cm", name="cm")
                nc.vector.tensor_scalar(
                    out=cm[:], in0=st[0][:], scalar1=1.0 / D, scalar2=None, op0=OP.mult
                )
                cmsq = wk.tile([1, 1], F32, tag="cmsq", name="cmsq")
                nc.vector.tensor_tensor(out=cmsq[:], in0=cm[:], in1=cm[:], op=OP.mult)
                cvar = wk.tile([1, 1], F32, tag="cvar", name="cvar")
                nc.vector.tensor_scalar(
                    out=cvar[:], in0=st[1][:], scalar1=1.0 / D, scalar2=None, op0=OP.mult
                )
                nc.vector.tensor_tensor(
                    out=cvar[:], in0=cvar[:], in1=cmsq[:], op=OP.subtract
                )
                csd = wk.tile([1, 1], F32, tag="csd", name="csd")
                nc.scalar.activation(csd[:], cvar[:], AF.Sqrt, bias=epsc[0:1])
                cr = wk.tile([1, 1], F32, tag="cr", name="cr")
                nc.vector.reciprocal(cr[:], csd[:])
                cm_b = wk.tile([128, 1], F32, tag="cmb", name="cmb")
                nc.gpsimd.partition_broadcast(cm_b[:], cm[:])
                cr_b = wk.tile([128, 1], F32, tag="crb", name="crb")
                nc.gpsimd.partition_broadcast(cr_b[:], cr[:])
                nc.vector.tensor_scalar(
                    out=hc[:], in0=pooled[:], scalar1=cm_b[:], scalar2=cr_b[:],
                    op0=OP.subtract, op1=OP.mult,
                )
                # c1 = tanh(Wc1' @ hc (+bc1))
                for mo in range(DC):
                    ps = pp.tile([128, 1], F32, tag="avtr", bufs=2, name="ps_c1")
                    wt = wpool.tile([128, DC, 128], BF16, tag="wqk",
                                    bufs=3, name="wc1")
                    nc.sync.dma_start(wt[:], wc1_d.ap()[mo])
                    pairs = [(wt[:, kc, :], hc[:, kc : kc + 1]) for kc in range(DC)]
                    if "c1" in bias_d:
                        pairs.append(
                            (bc1[:, 128 * mo : 128 * (mo + 1)], ones_row[:, :1])
                        )
                    mm_seq(ps[:], pairs)
                    nc.scalar.activation(c1[:, mo : mo + 1], ps[:], AF.Tanh)
                # out = Wc2 @ c1 (+bc2)
                ps = pp.tile([2, 1], F32, tag="avtr", bufs=2, name="ps_c2")
                for kc in range(DC):
                    nc.tensor.matmul(
                        ps[:], wc2[:, kc, :], c1[:, kc : kc + 1],
                        start=(kc == 0), stop=(kc == DC - 1),
                    )
                ores = wk.tile([2, 1], F32, tag="ores", name="ores")
                if "c2" in bias_d:
                    nc.vector.tensor_scalar(
                        out=ores[:], in0=ps[:], scalar1=bc2[:], scalar2=None, op0=OP.add
                    )
                else:
                    nc.vector.tensor_copy(ores[:], ps[:])
                nc.sync.dma_start(out_d.ap().rearrange("(a b) -> a b", b=1), ores[:])

    nc.finalize()
    return nc


def _host_prep(tokens, mask, emb, in_w, in_b, out_w, out_b, ln1_s, ln1_b, ln2_s,
               ln2_b, ff_w1, ff_b1, ff_w2, ff_b2, pool_w, pool_b, cls_ln_s,
               cls_ln_b, cls_w1, cls_b1, cls_w2, cls_b2):
    f32 = np.float32
    tokens = np.asarray(tokens).astype(np.int32)
    mask = np.asarray(mask).astype(np.int32)
    emb = np.ascontiguousarray(np.asarray(emb, f32))

    # positional encoding (match reference f32 math)
    pos = np.arange(S, dtype=f32)[:, None]
    div = np.exp(np.arange(0, D, 2, dtype=f32) * f32(-np.log(10000.0) / D))
    pe = np.zeros((S, D), f32)
    pe[:, 0::2] = np.sin(pos * div)
    pe[:, 1::2] = np.cos(pos * div)
    pe_t = np.ascontiguousarray(pe.T.reshape(DC, 128, S))

    def blocked(wT, nm):  # [din, dout] -> [nm, 128, DC, 128]: per-mo contiguous
        return np.ascontiguousarray(
            wT.reshape(DC, 128, nm, 128).transpose(2, 1, 0, 3).astype(nbf)
        )

    wqk_t = np.zeros((L, 12, 128, DC, 128), nbf)
    wv_t = np.zeros((L, 2, 128, DC, 384), nbf)
    wo_t = np.zeros((L, DC, 128, DC, 128), nbf)
    w1_t = np.zeros((L, FC, 128, DC, 128), nbf)
    w2_t = np.zeros((L, DC, 128, FC, 128), nbf)
    b_qk = np.zeros((L, 1, 1536), f32)
    b_v = np.zeros((L, 1, D), f32)
    b_o = np.zeros((L, 1, D), f32)
    b_f1 = np.zeros((L, 1, F), f32)
    b_f2 = np.zeros((L, 1, D), f32)
    for l in range(L):
        Wi = np.asarray(in_w[l], f32)
        bi = np.asarray(in_b[l], f32) + Wi @ np.asarray(ln1_b[l], f32)
        Wi = Wi * np.asarray(ln1_s[l], f32)[None, :]
        qs = f32(1.0 / np.sqrt(D // H))
        Wi = np.concatenate([Wi[:D] * qs, Wi[D:]], axis=0)
        bi = np.concatenate([bi[:D] * qs, bi[D:]], axis=0)
        wqk_t[l] = blocked(Wi[:1536].T, 12)
        wv_t[l] = np.ascontiguousarray(
            Wi[1536:].T.reshape(DC, 128, 2, 384).transpose(2, 1, 0, 3)
        ).astype(nbf)
        b_qk[l, 0] = bi[:1536]
        b_v[l, 0] = bi[1536:]
        wo_t[l] = blocked(np.asarray(out_w[l], f32).T, DC)
        b_o[l, 0] = np.asarray(out_b[l], f32)
        W1 = np.asarray(ff_w1[l], f32)
        b1 = np.asarray(ff_b1[l], f32) + W1 @ np.asarray(ln2_b[l], f32)
        W1 = W1 * np.asarray(ln2_s[l], f32)[None, :]
        w1_t[l] = blocked(W1.T, FC)
        b_f1[l, 0] = b1
        w2_t[l] = np.ascontiguousarray(
            np.asarray(ff_w2[l], f32).T.reshape(FC, 128, DC, 128)
            .transpose(2, 1, 0, 3).astype(nbf)
        )
        b_f2[l, 0] = np.asarray(ff_b2[l], f32)

    Wc1 = np.asarray(cls_w1, f32)
    bc1 = np.asarray(cls_b1, f32) + Wc1 @ np.asarray(cls_ln_b, f32)
    Wc1 = Wc1 * np.asarray(cls_ln_s, f32)[None, :]
    wc1_t = blocked(Wc1.T, DC)
    wc2_t = np.ascontiguousarray(
        np.asarray(cls_w2, f32).T.reshape(DC, 128, 2).astype(nbf)
    )
    b_c2 = np.asarray(cls_b2, f32).reshape(2, 1)
    wp_t = np.ascontiguousarray(
        np.asarray(pool_w, f32).reshape(D).reshape(DC, 128).T.astype(nbf)
    )  # [128, DC]

    # masks per core (band + key padding), token-major and transposed
    li = np.arange(128)[:, None]
    wcol = np.arange(W)[None, :]
    band = (wcol >= li) & (wcol <= li + 256)
    mask_tm = np.zeros((B, NB, 128, W), nbf)
    mask_tt = np.zeros((B, NB, 3, 128, 128), nbf)
    for bb in range(B):
        kpm = mask[bb] != 0
        for qb in range(NB):
            k_real = 128 * (qb - 1) + wcol
            in_range = (k_real >= 0) & (k_real < S)
            valid = band & in_range & kpm[np.clip(k_real, 0, S - 1)]
            m = np.where(valid, f32(0.0), f32(-30000.0))
            mask_tm[bb, qb] = m.astype(nbf)
            mask_tt[bb, qb] = np.ascontiguousarray(
                m.reshape(128, 3, 128).transpose(1, 2, 0)
            ).astype(nbf)

    penalty = (
        np.where(mask == 0, f32(-1e30), f32(0.0))
        + np.asarray(pool_b, f32).reshape(1)[0]
    ).astype(f32).reshape(B, 1, S)

    shared = dict(
        emb=emb, pe_t=pe_t,
        ident_f32=np.eye(128, dtype=f32),
        ident_bf16=np.eye(128, dtype=f32).astype(nbf),
        wqk_t=wqk_t, wv_t=wv_t, wo_t=wo_t, w1_t=w1_t, w2_t=w2_t,
        wc1_t=wc1_t, wc2_t=wc2_t, wp_t=wp_t,
    )
    maybe_bias = dict(
        qk=b_qk.astype(nbf), v=b_v.astype(nbf), o=b_o.astype(nbf),
        f1=b_f1.astype(nbf), f2=b_f2.astype(nbf),
        c1=bc1.astype(nbf).reshape(1, D), c2=b_c2,
    )
    apply_biases = {
        k: bool(np.any(np.asarray(v, f32) != 0)) for k, v in maybe_bias.items()
    }
    bias_inputs = {
        "b_" + k: v for k, v in maybe_bias.items() if apply_biases[k]
    }
    per_core = [
        dict(tokens=tokens[bb], penalty=penalty[bb],
             mask_tm=mask_tm[bb], mask_tt=mask_tt[bb])
        for bb in range(B)
    ]
    return shared, bias_inputs, apply_biases, per_core


def kernel(**inputs):
    global LAST_RESULTS
    shared, bias_inputs, apply_biases, per_core = _host_prep(**inputs)

    key = tuple(sorted(apply_biases.items()))
    if key not in _CACHE:
        _CACHE[key] = _build_nc(apply_biases)
    nc = _CACHE[key]

    in_maps = []
    for bb in range(B):
        m = dict(shared)
        m.update(bias_inputs)
        m.update(per_core[bb])
        in_maps.append(m)

    res = run_bass_kernel_spmd(nc, in_maps, list(range(B)), trace=False)
    LAST_RESULTS = res

    out = np.stack([np.asarray(res.results[bb]["cls_out"]) for bb in range(B)])
    all_w = np.stack([np.asarray(res.results[bb]["all_w"]) for bb in range(B)])
    all_w = np.ascontiguousarray(all_w.transpose(1, 0, 2, 3))  # [L, B, S, S]
    return out.astype(np.float32), all_w.astype(np.float32)


def time_kernel(reps=20, **inputs):
    """Repeat-execute the compiled NEFF (non-donated buffers, inputs resident
    on device) and return per-call wall seconds. Mirrors
    bass2jax.run_bass_via_pjrt's sharded setup minus donation."""
    import time
    import jax
    import concourse.mybir as mybir_
    from concourse import bass2jax
    from jax.experimental.shard_map import shard_map
    from jax.sharding import Mesh, PartitionSpec

    shared, bias_inputs, apply_biases, per_core = _host_prep(**inputs)
    key = tuple(sorted(apply_biases.items()))
    if key not in _CACHE:
        _CACHE[key] = _build_nc(apply_biases)
    nc = _CACHE[key]
    in_maps = []
    for bb in range(B):
        m = dict(shared)
        m.update(bias_inputs)
        m.update(per_core[bb])
        in_maps.append(m)

    bass2jax.install_neuronx_cc_hook()
    n_cores = B
    in_names, out_names, out_avals, zero_outs = [], [], [], []
    for alloc in nc.m.functions[0].allocations:
        if not isinstance(alloc, mybir_.MemoryLocationSet):
            continue
        name = alloc.memorylocations[0].name
        if alloc.kind == "ExternalInput":
            in_names.append(name)
        elif alloc.kind == "ExternalOutput":
            out_names.append(name)
            shape = tuple(alloc.tensor_shape)
            dtype = mybir_.dt.np(alloc.dtype)
            out_avals.append(jax.core.ShapedArray(shape, dtype))
            zero_outs.append(np.zeros(shape, dtype))
    partition_name = (
        nc.partition_id_tensor.name if nc.partition_id_tensor else None
    )
    if partition_name in in_names:
        in_names.remove(partition_name)
    n_params = len(in_names)
    all_in_names = in_names + out_names
    if partition_name is not None:
        all_in_names.append(partition_name)

    def _body(*args):
        operands = list(args)
        if partition_name is not None:
            operands.append(bass2jax.partition_id_tensor())
        outs = bass2jax._bass_exec_p.bind(
            *operands,
            out_avals=tuple(out_avals),
            in_names=tuple(all_in_names),
            out_names=tuple(out_names),
            lowering_input_output_aliases=(),
            sim_require_finite=True,
            sim_require_nnan=True,
            nc=nc,
        )
        return tuple(outs)

    devices = jax.devices()[:n_cores]
    mesh = Mesh(np.asarray(devices), ("core",))
    n_outs = len(out_names)
    sharded = jax.jit(
        shard_map(
            _body, mesh=mesh,
            in_specs=(PartitionSpec("core"),) * (n_params + n_outs),
            out_specs=(PartitionSpec("core"),) * n_outs,
            check_rep=False,
        ),
        keep_unused=True,
    )
    concat_in = [
        np.concatenate([np.asarray(in_maps[c][nm]) for c in range(n_cores)], axis=0)
        for nm in in_names
    ]
    concat_zeros = [
        np.zeros((n_cores * z.shape[0], *z.shape[1:]), z.dtype) for z in zero_outs
    ]
    from jax.sharding import NamedSharding
    args = [
        jax.device_put(a, NamedSharding(mesh, PartitionSpec("core")))
        for a in concat_in + concat_zeros
    ]
    # warm-up (compile + first exec)
    r = sharded(*args)
    jax.block_until_ready(r)
    times = []
    for _ in range(3):
        t0 = time.perf_counter()
        for _ in range(reps):
            r = sharded(*args)
        jax.block_until_ready(r)
        times.append((time.perf_counter() - t0) / reps)
    return times


def time_kernel_batched(n_lo=2, n_hi=12, rounds=3, **inputs):
    """Time N sequential NEFF executions inside one jitted call; the slope
    (t_hi - t_lo)/(n_hi - n_lo) removes per-dispatch tunnel overhead."""
    import time
    import jax
    import concourse.mybir as mybir_
    from concourse import bass2jax
    from jax.experimental.shard_map import shard_map
    from jax.sharding import Mesh, NamedSharding, PartitionSpec

    shared, bias_inputs, apply_biases, per_core = _host_prep(**inputs)
    key = tuple(sorted(apply_biases.items()))
    if key not in _CACHE:
        _CACHE[key] = _build_nc(apply_biases)
    nc = _CACHE[key]
    in_maps = []
    for bb in range(B):
        m = dict(shared)
        m.update(bias_inputs)
        m.update(per_core[bb])
        in_maps.append(m)

    bass2jax.install_neuronx_cc_hook()
    n_cores = B
    in_names, out_names, out_avals, zero_outs = [], [], [], []
    for alloc in nc.m.functions[0].allocations:
        if not isinstance(alloc, mybir_.MemoryLocationSet):
            continue
        name = alloc.memorylocations[0].name
        if alloc.kind == "ExternalInput":
            in_names.append(name)
        elif alloc.kind == "ExternalOutput":
            out_names.append(name)
            shape = tuple(alloc.tensor_shape)
            dtype = mybir_.dt.np(alloc.dtype)
            out_avals.append(jax.core.ShapedArray(shape, dtype))
            zero_outs.append(np.zeros(shape, dtype))
    partition_name = (
        nc.partition_id_tensor.name if nc.partition_id_tensor else None
    )
    if partition_name in in_names:
        in_names.remove(partition_name)
    n_params = len(in_names)
    all_in_names = in_names + out_names
    if partition_name is not None:
        all_in_names.append(partition_name)

    def make_body(n):
        def _body(*args):
            outs = None
            for _ in range(n):
                operands = list(args)
                if partition_name is not None:
                    operands.append(bass2jax.partition_id_tensor())
                outs = bass2jax._bass_exec_p.bind(
                    *operands,
                    out_avals=tuple(out_avals),
                    in_names=tuple(all_in_names),
                    out_names=tuple(out_names),
                    lowering_input_output_aliases=(),
                    sim_require_finite=True,
                    sim_require_nnan=True,
                    nc=nc,
                )
            return tuple(outs)
        return _body

    devices = jax.devices()[:n_cores]
    mesh = Mesh(np.asarray(devices), ("core",))
    n_outs = len(out_names)
    concat_in = [
        np.concatenate([np.asarray(in_maps[c][nm]) for c in range(n_cores)], axis=0)
        for nm in in_names
    ]
    concat_zeros = [
        np.zeros((n_cores * z.shape[0], *z.shape[1:]), z.dtype) for z in zero_outs
    ]
    args = [
        jax.device_put(a, NamedSharding(mesh, PartitionSpec("core")))
        for a in concat_in + concat_zeros
    ]
    results = {}
    for n in (n_lo, n_hi):
        f = jax.jit(
            shard_map(
                make_body(n), mesh=mesh,
                in_specs=(PartitionSpec("core"),) * (n_params + n_outs),
                out_specs=(PartitionSpec("core"),) * n_outs,
                check_rep=False,
            ),
            keep_unused=True,
        )
        r = f(*args)
        jax.block_until_ready(r)
        best = 1e9
        for _ in range(rounds):
            t0 = time.perf_counter()
            r = f(*args)
            jax.block_until_ready(r)
            best = min(best, time.perf_counter() - t0)
        results[n] = best
    slope = (results[n_hi] - results[n_lo]) / (n_hi - n_lo)
    return slope, results
